# revision 16
# baseline (speedup 1.0000x reference)
"""Trainium2 Bass kernel for nn_Mirror: per-sample conditional flips + fp16 cast.

Full op: x [16,2,64,128,128] f32, x_flag [16], y_flag [16] f32 ->
out [16,2,64,128,128] f16 where per sample b:
  out[b] = 0                 if x_flag[b] <= 0.5
         = flip_h(x[b])      if x_flag[b] > 0.5 and y_flag[b] <= 0.5
         = flip_hw(x[b])     if x_flag[b] > 0.5 and y_flag[b] > 0.5

A sample [2,64,128,128] is 128 images of 128x128; images map onto the 128
SBUF partitions, so sample data is a [128, 16384] block whose flips are pure
free-dim manipulations: flip_h reverses within each 128-elem row; flip_w
reverses the order of the 128-elem rows (and full flip_hw maps w-block u of
the input to w-block 127-u of the output).

Device program (identical on all 8 cores): 16 independent UNIT slots, each a
[128, 2048] f32 block (1/8 of a sample = 16 w-rows of every image):
    load  T <- x_unit[u]           (sync HWDGE,   cond = act[u] != 0)
    O = revh(T) cast fp16          (single 1-input DVE pass)
    store out_unit[u] <- O         (gpsimd SWDGE, cond = a[u] != 0)  OR
    store out_unit[u] <- revw(O)   (gpsimd SWDGE, cond = b[u] != 0;
                                    w-block reversal on the SBUF read AP)
Flags a/b/act are host-encoded 0.0/1.0 per unit; conds compare raw float
bits in engine registers.  Skipped DMAs still bump their semaphores, so the
Tile schedule is flag-oblivious.  Inactive units move zero bytes.

Host side: only ACTIVE samples (x_flag > 0.5) produce units; their 8 units
are dealt round-robin over cores (8k units spread as k per core -> perfect
balance for any k, worst case 16 units/core = 2 full samples).  For y-flipped
samples the host ships unit u and places the device result at unit 7-u of the
output; inactive samples are host-side zeros (device output DRAM is also
pre-zeroed, but never read for them).
"""

import numpy as np

import concourse.bass as bass
import concourse.mybir as mybir
import concourse.tile as tile
from concourse import bacc
from concourse.bass_utils import run_bass_kernel_spmd
from concourse.ordered_set import OrderedSet

N_CORES = 8
FULL_B = 16
IMG = 128              # images per sample = SBUF partitions
WH = 16384             # free elems per image
CH = 2048              # unit width (16 w-rows)
UPS = WH // CH         # units per sample (8)
U = 16                 # unit slots per core
H = 128

SP = mybir.EngineType.SP
POOL = mybir.EngineType.Pool


def build_program(sim_init=False):
    nc = bacc.Bacc("TRN2", target_bir_lowering=False, debug=False)
    x = nc.dram_tensor("x", [U, IMG, CH], mybir.dt.float32, kind="ExternalInput")
    af = nc.dram_tensor("a_flag", [U], mybir.dt.float32, kind="ExternalInput")
    bf = nc.dram_tensor("b_flag", [U], mybir.dt.float32, kind="ExternalInput")
    out = nc.dram_tensor("out", [U, IMG, CH], mybir.dt.float16, kind="ExternalOutput")

    with tile.TileContext(nc) as tc:
        with (
            tc.tile_pool(name="flags", bufs=1) as flag_pool,
            tc.tile_pool(name="in", bufs=12) as in_pool,
            tc.tile_pool(name="out", bufs=10) as out_pool,
        ):
            fa = flag_pool.tile([1, U], mybir.dt.float32, tag="fa")
            fb = flag_pool.tile([1, U], mybir.dt.float32, tag="fb")
            nc.sync.dma_start(fa[:], af.ap().unsqueeze(0))
            nc.sync.dma_start(fb[:], bf.ap().unsqueeze(0))
            fa_i = fa[:].bitcast(mybir.dt.int32)
            fb_i = fb[:].bitcast(mybir.dt.int32)

            for u in range(U):
                # per-unit conds on the DMA-issuing engines; bits(1.0) != 0
                ar = nc.alloc_registers(f"ar{u}", engines=OrderedSet([SP, POOL]))
                br = nc.alloc_registers(f"br{u}", engines=OrderedSet([SP, POOL]))
                nc.regs_load(ar, fa_i[0:1, u : u + 1])
                nc.regs_load(br, fb_i[0:1, u : u + 1])
                act = (
                    nc.snap(ar, engines=OrderedSet([SP]))
                    + nc.snap(br, engines=OrderedSet([SP]))
                ) != 0
                c_a = nc.snap(ar, engines=OrderedSet([POOL])) != 0
                c_b = nc.snap(br, engines=OrderedSet([POOL])) != 0

                t = in_pool.tile([IMG, CH], mybir.dt.float32, tag="tin")
                if sim_init:
                    # CoreSim-only: skipped loads leave tiles uninit, which
                    # the sim rejects; HW reads garbage that is never stored.
                    nc.gpsimd.memset(t[:], 0.0)
                nc.sync.dma_start(t[:], x.ap()[u], cond=act)

                og = out_pool.tile([IMG, CH], mybir.dt.float16, tag="og")
                # O = revh(T): reverse within each 128-elem image row + cast
                src = t[:].rearrange("p (w h) -> p w h", h=H)[:, :, ::-1]
                dst = og[:].rearrange("p (w h) -> p w h", h=H)
                nc.vector.tensor_copy(dst, src)

                # exactly one of these fires for an active unit
                nc.gpsimd.dma_start(out.ap()[u], og[:], cond=c_a)
                og_wrev = og[:].rearrange("p (w h) -> p w h", h=H)[:, ::-1, :]
                nc.gpsimd.dma_start(out.ap()[u], og_wrev, cond=c_b)
    nc.compile()
    return nc


_NC_CACHE = None


def _get_program():
    global _NC_CACHE
    if _NC_CACHE is None:
        _NC_CACHE = build_program()
    return _NC_CACHE


def kernel(x, x_flag, y_flag, _trace=False, **trace_kwargs):
    x = np.asarray(x)
    if x.dtype != np.float32:
        x = x.astype(np.float32)
    x_flag = np.asarray(x_flag, dtype=np.float32)
    y_flag = np.asarray(y_flag, dtype=np.float32)
    n = x.shape[0]
    assert n == FULL_B, x.shape
    sample_shape = x.shape[1:]
    xv = x.reshape(n, IMG, WH)

    # host schedule: 8 units per active sample, dealt round-robin over cores
    active = [int(i) for i in np.nonzero(x_flag > 0.5)[0]]
    units = [(idx, u) for idx in active for u in range(UPS)]
    assert len(units) <= N_CORES * U

    xs = np.zeros((N_CORES, U, IMG, CH), dtype=np.float32)
    afs = np.zeros((N_CORES, U), dtype=np.float32)
    bfs = np.zeros((N_CORES, U), dtype=np.float32)
    placement = []  # (core, slot, sample, out_unit)
    for i, (idx, u) in enumerate(units):
        c, s = i % N_CORES, i // N_CORES
        xs[c, s] = xv[idx, :, u * CH : (u + 1) * CH]
        yflip = y_flag[idx] > 0.5
        if yflip:
            bfs[c, s] = 1.0
            placement.append((c, s, idx, UPS - 1 - u))
        else:
            afs[c, s] = 1.0
            placement.append((c, s, idx, u))

    in_maps = [
        {"x": xs[c], "a_flag": afs[c], "b_flag": bfs[c]} for c in range(N_CORES)
    ]
    nc = _get_program()
    res = run_bass_kernel_spmd(
        nc, in_maps, core_ids=list(range(N_CORES)), trace=_trace, **trace_kwargs
    )

    out = np.zeros((n, IMG, WH), dtype=np.float16)
    for c, s, idx, pu in placement:
        out[idx, :, pu * CH : (pu + 1) * CH] = res.results[c]["out"][s]
    out = out.reshape((n,) + sample_shape)
    if _trace:
        return out, res
    return out


# revision 18
# speedup vs baseline: 1.0424x; 1.0424x over previous
"""Trainium2 Bass kernel for nn_Mirror: per-sample conditional flips + fp16 cast.

Full op: x [16,2,64,128,128] f32, x_flag [16], y_flag [16] f32 ->
out [16,2,64,128,128] f16 where per sample b:
  out[b] = 0                 if x_flag[b] <= 0.5
         = flip_h(x[b])      if x_flag[b] > 0.5 and y_flag[b] <= 0.5
         = flip_hw(x[b])     if x_flag[b] > 0.5 and y_flag[b] > 0.5

A sample [2,64,128,128] is 128 images of 128x128; images map onto the 128
SBUF partitions, so sample data is a [128, 16384] block whose flips are pure
free-dim manipulations: flip_h reverses within each 128-elem row; flip_w
reverses the order of the 128-elem rows (and full flip_hw maps w-block u of
the input to w-block 127-u of the output).

Device program (identical on all 8 cores): 16 independent UNIT slots, each a
[128, 2048] f32 block (1/8 of a sample = 16 w-rows of every image):
    load  T <- x_unit[u]           (sync HWDGE,   cond = act[u] != 0)
    O = revh(T) cast fp16          (single 1-input DVE pass)
    store out_unit[u] <- O         (gpsimd SWDGE, cond = a[u] != 0)  OR
    store out_unit[u] <- revw(O)   (gpsimd SWDGE, cond = b[u] != 0;
                                    w-block reversal on the SBUF read AP)
Flags a/b/act are host-encoded 0.0/1.0 per unit; conds compare raw float
bits in engine registers.  Skipped DMAs still bump their semaphores, so the
Tile schedule is flag-oblivious.  Inactive units move zero bytes.

Host side: only ACTIVE samples (x_flag > 0.5) produce units; their 8 units
are dealt round-robin over cores (8k units spread as k per core -> perfect
balance for any k, worst case 16 units/core = 2 full samples).  For y-flipped
samples the host ships unit u and places the device result at unit 7-u of the
output; inactive samples are host-side zeros (device output DRAM is also
pre-zeroed, but never read for them).
"""

import numpy as np

import concourse.bass as bass
import concourse.mybir as mybir
import concourse.tile as tile
from concourse import bacc
from concourse.bass_utils import run_bass_kernel_spmd
from concourse.ordered_set import OrderedSet

N_CORES = 8
FULL_B = 16
IMG = 128              # images per sample = SBUF partitions
WH = 16384             # free elems per image
CH = 2048              # unit width (16 w-rows)
UPS = WH // CH         # units per sample (8)
U = 16                 # unit slots per core
H = 128

SP = mybir.EngineType.SP
POOL = mybir.EngineType.Pool


def build_program(sim_init=False):
    nc = bacc.Bacc("TRN2", target_bir_lowering=False, debug=False)
    x = nc.dram_tensor("x", [U, IMG, CH], mybir.dt.float32, kind="ExternalInput")
    # bit-packed unit flags: [act_mask, a_mask, b_mask], bit u = unit u
    fm = nc.dram_tensor("flag_masks", [3], mybir.dt.int32, kind="ExternalInput")
    out = nc.dram_tensor("out", [U, IMG, CH], mybir.dt.float16, kind="ExternalOutput")

    with tile.TileContext(nc) as tc:
        with (
            tc.tile_pool(name="flags", bufs=1) as flag_pool,
            tc.tile_pool(name="in", bufs=12) as in_pool,
            tc.tile_pool(name="out", bufs=10) as out_pool,
        ):
            fmt = flag_pool.tile([1, 3], mybir.dt.int32, tag="fm")
            nc.sync.dma_start(fmt[:], fm.ap().unsqueeze(0))

            # one mask register per issuing engine; per-unit conds are pure
            # ALU on the snapped value (no per-unit loads or sem waits)
            actr = nc.alloc_registers("actr", engines=OrderedSet([SP]))
            ar = nc.alloc_registers("ar", engines=OrderedSet([POOL]))
            br = nc.alloc_registers("br", engines=OrderedSet([POOL]))
            nc.regs_load(actr, fmt[0:1, 0:1])
            nc.regs_load(ar, fmt[0:1, 1:2])
            nc.regs_load(br, fmt[0:1, 2:3])
            act_m = nc.snap(actr, engines=OrderedSet([SP]), min_val=0, max_val=(1 << U) - 1)
            a_m = nc.snap(ar, engines=OrderedSet([POOL]), min_val=0, max_val=(1 << U) - 1)
            b_m = nc.snap(br, engines=OrderedSet([POOL]), min_val=0, max_val=(1 << U) - 1)

            for u in range(U):
                act = (act_m & (1 << u)) != 0
                c_a = (a_m & (1 << u)) != 0
                c_b = (b_m & (1 << u)) != 0

                t = in_pool.tile([IMG, CH], mybir.dt.float32, tag="tin")
                if sim_init:
                    # CoreSim-only: skipped loads leave tiles uninit, which
                    # the sim rejects; HW reads garbage that is never stored.
                    nc.gpsimd.memset(t[:], 0.0)
                nc.sync.dma_start(t[:], x.ap()[u], cond=act)

                og = out_pool.tile([IMG, CH], mybir.dt.float16, tag="og")
                # O = revh(T): reverse within each 128-elem image row + cast
                src = t[:].rearrange("p (w h) -> p w h", h=H)[:, :, ::-1]
                dst = og[:].rearrange("p (w h) -> p w h", h=H)
                nc.vector.tensor_copy(dst, src)

                # exactly one of these fires for an active unit
                nc.gpsimd.dma_start(out.ap()[u], og[:], cond=c_a)
                og_wrev = og[:].rearrange("p (w h) -> p w h", h=H)[:, ::-1, :]
                nc.gpsimd.dma_start(out.ap()[u], og_wrev, cond=c_b)
    nc.compile()
    return nc


_NC_CACHE = None


def _get_program():
    global _NC_CACHE
    if _NC_CACHE is None:
        _NC_CACHE = build_program()
    return _NC_CACHE


def kernel(x, x_flag, y_flag, _trace=False, **trace_kwargs):
    x = np.asarray(x)
    if x.dtype != np.float32:
        x = x.astype(np.float32)
    x_flag = np.asarray(x_flag, dtype=np.float32)
    y_flag = np.asarray(y_flag, dtype=np.float32)
    n = x.shape[0]
    assert n == FULL_B, x.shape
    sample_shape = x.shape[1:]
    xv = x.reshape(n, IMG, WH)

    # host schedule: 8 units per active sample, dealt round-robin over cores
    active = [int(i) for i in np.nonzero(x_flag > 0.5)[0]]
    units = [(idx, u) for idx in active for u in range(UPS)]
    assert len(units) <= N_CORES * U

    xs = np.zeros((N_CORES, U, IMG, CH), dtype=np.float32)
    masks = np.zeros((N_CORES, 3), dtype=np.int32)  # act, a, b bit-masks
    placement = []  # (core, slot, sample, out_unit)
    for i, (idx, u) in enumerate(units):
        c, s = i % N_CORES, i // N_CORES
        xs[c, s] = xv[idx, :, u * CH : (u + 1) * CH]
        masks[c, 0] |= 1 << s
        yflip = y_flag[idx] > 0.5
        if yflip:
            masks[c, 2] |= 1 << s
            placement.append((c, s, idx, UPS - 1 - u))
        else:
            masks[c, 1] |= 1 << s
            placement.append((c, s, idx, u))

    in_maps = [{"x": xs[c], "flag_masks": masks[c]} for c in range(N_CORES)]
    nc = _get_program()
    res = run_bass_kernel_spmd(
        nc, in_maps, core_ids=list(range(N_CORES)), trace=_trace, **trace_kwargs
    )

    out = np.zeros((n, IMG, WH), dtype=np.float16)
    for c, s, idx, pu in placement:
        out[idx, :, pu * CH : (pu + 1) * CH] = res.results[c]["out"][s]
    out = out.reshape((n,) + sample_shape)
    if _trace:
        return out, res
    return out


# revision 19
# speedup vs baseline: 1.1666x; 1.1192x over previous
"""Trainium2 Bass kernel for nn_Mirror: per-sample conditional flips + fp16 cast.

Full op: x [16,2,64,128,128] f32, x_flag [16], y_flag [16] f32 ->
out [16,2,64,128,128] f16 where per sample b:
  out[b] = 0                 if x_flag[b] <= 0.5
         = flip_h(x[b])      if x_flag[b] > 0.5 and y_flag[b] <= 0.5
         = flip_hw(x[b])     if x_flag[b] > 0.5 and y_flag[b] > 0.5

A sample [2,64,128,128] is 128 images of 128x128; images map onto the 128
SBUF partitions, so a sample is a [128, 16384] block whose flips are pure
free-dim manipulations: flip_h reverses within each 128-elem image row;
flip_hw additionally maps w-row block q of the input to block 127-q of the
output.

Device program (identical on all 8 cores): 8 independent UNIT slots, each a
contiguous [128, 4096] f32 block (a quarter sample = 32 w-rows of every
image):
    load  T <- x_unit[u]           (sync HWDGE,   cond = act bit u)
    O = revh(T) cast fp16          (single 1-input DVE pass)
    store out_unit[u] <- O         (gpsimd SWDGE, cond = a bit u)   OR
    store out_unit[u] <- revw(O)   (gpsimd SWDGE, cond = b bit u;
                                    w-block reversal on the SBUF read AP)
Unit flags arrive bit-packed in one int32 per mask (act/a/b), loaded once
into one register per DMA-issuing engine; per-unit conds are pure register
ALU.  Skipped DMAs still bump their semaphores, so the Tile schedule is
flag-oblivious.  Inactive units move zero bytes.

Host side: only ACTIVE samples (x_flag > 0.5) produce units; their 4
quarter-units are dealt round-robin over cores (4k units spread over 8
cores; worst case k=16 -> 8 units/core = 2 full samples).  For y-flipped
samples the host ships quarter q and places the device result at quarter
3-q of the output; inactive samples are host-side zeros.
"""

import numpy as np

import concourse.bass as bass
import concourse.mybir as mybir
import concourse.tile as tile
from concourse import bacc
from concourse.bass_utils import run_bass_kernel_spmd
from concourse.ordered_set import OrderedSet

N_CORES = 8
FULL_B = 16
IMG = 128              # images per sample = SBUF partitions
WH = 16384             # free elems per image
CH = 4096              # unit width (32 w-rows)
UPS = WH // CH         # units per sample (4)
U = 8                  # unit slots per core
H = 128

SP = mybir.EngineType.SP
POOL = mybir.EngineType.Pool


def build_program(sim_init=False):
    nc = bacc.Bacc("TRN2", target_bir_lowering=False, debug=False)
    x = nc.dram_tensor("x", [U, IMG, CH], mybir.dt.float32, kind="ExternalInput")
    # bit-packed unit flags: [act_mask, a_mask, b_mask], bit u = unit u
    fm = nc.dram_tensor("flag_masks", [3], mybir.dt.int32, kind="ExternalInput")
    out = nc.dram_tensor("out", [U, IMG, CH], mybir.dt.float16, kind="ExternalOutput")

    with tile.TileContext(nc) as tc:
        with (
            tc.tile_pool(name="flags", bufs=1) as flag_pool,
            tc.tile_pool(name="in", bufs=6) as in_pool,
            tc.tile_pool(name="out", bufs=5) as out_pool,
        ):
            fmt = flag_pool.tile([1, 3], mybir.dt.int32, tag="fm")
            nc.sync.dma_start(fmt[:], fm.ap().unsqueeze(0))

            # one mask register per issuing engine; per-unit conds are pure
            # ALU on the snapped value (no per-unit loads or sem waits)
            actr = nc.alloc_registers("actr", engines=OrderedSet([SP]))
            ar = nc.alloc_registers("ar", engines=OrderedSet([POOL]))
            br = nc.alloc_registers("br", engines=OrderedSet([POOL]))
            nc.regs_load(actr, fmt[0:1, 0:1])
            nc.regs_load(ar, fmt[0:1, 1:2])
            nc.regs_load(br, fmt[0:1, 2:3])
            lim = (1 << U) - 1
            act_m = nc.snap(actr, engines=OrderedSet([SP]), min_val=0, max_val=lim)
            a_m = nc.snap(ar, engines=OrderedSet([POOL]), min_val=0, max_val=lim)
            b_m = nc.snap(br, engines=OrderedSet([POOL]), min_val=0, max_val=lim)

            for u in range(U):
                act = (act_m & (1 << u)) != 0
                c_a = (a_m & (1 << u)) != 0
                c_b = (b_m & (1 << u)) != 0

                t = in_pool.tile([IMG, CH], mybir.dt.float32, tag="tin")
                if sim_init:
                    # CoreSim-only: skipped loads leave tiles uninit, which
                    # the sim rejects; HW reads garbage that is never stored.
                    nc.gpsimd.memset(t[:], 0.0)
                nc.sync.dma_start(t[:], x.ap()[u], cond=act)

                og = out_pool.tile([IMG, CH], mybir.dt.float16, tag="og")
                # O = revh(T): reverse within each 128-elem image row + cast
                src = t[:].rearrange("p (w h) -> p w h", h=H)[:, :, ::-1]
                dst = og[:].rearrange("p (w h) -> p w h", h=H)
                nc.vector.tensor_copy(dst, src)

                # exactly one of these fires for an active unit
                nc.gpsimd.dma_start(out.ap()[u], og[:], cond=c_a)
                og_wrev = og[:].rearrange("p (w h) -> p w h", h=H)[:, ::-1, :]
                nc.gpsimd.dma_start(out.ap()[u], og_wrev, cond=c_b)
    nc.compile()
    return nc


_NC_CACHE = None


def _get_program():
    global _NC_CACHE
    if _NC_CACHE is None:
        _NC_CACHE = build_program()
    return _NC_CACHE


def kernel(x, x_flag, y_flag, _trace=False, **trace_kwargs):
    x = np.asarray(x)
    if x.dtype != np.float32:
        x = x.astype(np.float32)
    x_flag = np.asarray(x_flag, dtype=np.float32)
    y_flag = np.asarray(y_flag, dtype=np.float32)
    n = x.shape[0]
    assert n == FULL_B, x.shape
    sample_shape = x.shape[1:]
    xv = x.reshape(n, IMG, WH)

    # host schedule: 4 quarter-units per active sample, dealt round-robin
    active = [int(i) for i in np.nonzero(x_flag > 0.5)[0]]
    units = [(idx, u) for idx in active for u in range(UPS)]
    assert len(units) <= N_CORES * U

    xs = np.zeros((N_CORES, U, IMG, CH), dtype=np.float32)
    masks = np.zeros((N_CORES, 3), dtype=np.int32)  # act, a, b bit-masks
    placement = []  # (core, slot, sample, out_unit)
    for i, (idx, u) in enumerate(units):
        c, s = i % N_CORES, i // N_CORES
        xs[c, s] = xv[idx, :, u * CH : (u + 1) * CH]
        masks[c, 0] |= 1 << s
        if y_flag[idx] > 0.5:
            masks[c, 2] |= 1 << s
            placement.append((c, s, idx, UPS - 1 - u))
        else:
            masks[c, 1] |= 1 << s
            placement.append((c, s, idx, u))

    in_maps = [{"x": xs[c], "flag_masks": masks[c]} for c in range(N_CORES)]
    nc = _get_program()
    res = run_bass_kernel_spmd(
        nc, in_maps, core_ids=list(range(N_CORES)), trace=_trace, **trace_kwargs
    )

    out = np.zeros((n, IMG, WH), dtype=np.float16)
    for c, s, idx, pu in placement:
        out[idx, :, pu * CH : (pu + 1) * CH] = res.results[c]["out"][s]
    out = out.reshape((n,) + sample_shape)
    if _trace:
        return out, res
    return out


# revision 24
# speedup vs baseline: 1.2638x; 1.0833x over previous
"""Trainium2 Bass kernel for nn_Mirror: per-sample conditional flips + fp16 cast.

Full op: x [16,2,64,128,128] f32, x_flag [16], y_flag [16] f32 ->
out [16,2,64,128,128] f16 where per sample b:
  out[b] = 0                 if x_flag[b] <= 0.5
         = flip_h(x[b])      if x_flag[b] > 0.5 and y_flag[b] <= 0.5
         = flip_hw(x[b])     if x_flag[b] > 0.5 and y_flag[b] > 0.5

A sample [2,64,128,128] is 128 images of 128x128; images map onto the 128
SBUF partitions, so a sample is a [128, 16384] block whose flips are pure
free-dim manipulations: flip_h reverses within each 128-elem image row;
flip_hw additionally maps w-row block q of the input to block 127-q of the
output.

Device program (identical on all 8 cores): 8 independent UNIT slots, each a
contiguous [128, 4096] f32 block (a quarter sample = 32 w-rows of every
image):
    load  T <- x_unit[u]           (sync HWDGE,   cond = act bit u)
    O = revh(T) cast fp16          (single 1-input DVE pass)
    store out_unit[u] <- O         (gpsimd SWDGE, cond = a bit u)   OR
    store out_unit[u] <- revw(O)   (gpsimd SWDGE, cond = b bit u;
                                    w-block reversal on the SBUF read AP)
Unit flags arrive bit-packed in one int32 per mask (act/a/b), loaded once
into one register per DMA-issuing engine; per-unit conds are pure register
ALU.  Skipped DMAs still bump their semaphores, so the Tile schedule is
flag-oblivious.  Inactive units move zero bytes.

Host side: only ACTIVE samples (x_flag > 0.5) produce units; their 4
quarter-units are dealt round-robin over cores (4k units spread over 8
cores; worst case k=16 -> 8 units/core = 2 full samples).  For y-flipped
samples the host ships quarter q and places the device result at quarter
3-q of the output; inactive samples are host-side zeros.
"""

import numpy as np

import concourse.bass as bass
import concourse.mybir as mybir
import concourse.tile as tile
from concourse import bacc
from concourse.bass_utils import run_bass_kernel_spmd
from concourse.ordered_set import OrderedSet

N_CORES = 8
FULL_B = 16
IMG = 128              # images per sample = SBUF partitions
WH = 16384             # free elems per image
CH = 4096              # unit width (32 w-rows)
UPS = WH // CH         # units per sample (4)
U = 8                  # unit slots per core
H = 128

SP = mybir.EngineType.SP
POOL = mybir.EngineType.Pool


def build_program(sim_init=False):
    nc = bacc.Bacc("TRN2", target_bir_lowering=False, debug=False)
    x = nc.dram_tensor("x", [U, IMG, CH], mybir.dt.float32, kind="ExternalInput")
    # bit-packed unit flags: [act_mask, a_mask, b_mask], bit u = unit u
    fm = nc.dram_tensor("flag_masks", [3], mybir.dt.int32, kind="ExternalInput")
    # two disjoint regions (A: no w-flip, B: w-flipped) so the per-unit
    # conditional stores have no DRAM write-write overlap to serialize on;
    # the host reads whichever region the unit's flag selected
    out = nc.dram_tensor(
        "out", [2, U, IMG, CH], mybir.dt.float16, kind="ExternalOutput"
    )

    with tile.TileContext(nc) as tc:
        with (
            tc.tile_pool(name="flags", bufs=1) as flag_pool,
            tc.tile_pool(name="in", bufs=7) as in_pool,
            tc.tile_pool(name="out", bufs=6) as out_pool,
        ):
            fmt = flag_pool.tile([1, 3], mybir.dt.int32, tag="fm")
            nc.sync.dma_start(fmt[:], fm.ap().unsqueeze(0))

            # one mask register per issuing engine; per-unit conds are pure
            # ALU on the snapped value (no per-unit loads or sem waits)
            actr = nc.alloc_registers("actr", engines=OrderedSet([SP]))
            ar = nc.alloc_registers("ar", engines=OrderedSet([POOL]))
            br = nc.alloc_registers("br", engines=OrderedSet([POOL]))
            nc.regs_load(actr, fmt[0:1, 0:1])
            nc.regs_load(ar, fmt[0:1, 1:2])
            nc.regs_load(br, fmt[0:1, 2:3])
            lim = (1 << U) - 1
            act_m = nc.snap(actr, engines=OrderedSet([SP]), min_val=0, max_val=lim)
            a_m = nc.snap(ar, engines=OrderedSet([POOL]), min_val=0, max_val=lim)
            b_m = nc.snap(br, engines=OrderedSet([POOL]), min_val=0, max_val=lim)

            for u in range(U):
                act = (act_m & (1 << u)) != 0
                c_a = (a_m & (1 << u)) != 0
                c_b = (b_m & (1 << u)) != 0

                t = in_pool.tile([IMG, CH], mybir.dt.float32, tag="tin")
                if sim_init:
                    # CoreSim-only: skipped loads leave tiles uninit, which
                    # the sim rejects; HW reads garbage that is never stored.
                    nc.gpsimd.memset(t[:], 0.0)
                nc.sync.dma_start(t[:], x.ap()[u], cond=act)

                og = out_pool.tile([IMG, CH], mybir.dt.float16, tag="og")
                # O = revh(T): reverse within each 128-elem image row + cast
                src = t[:].rearrange("p (w h) -> p w h", h=H)[:, :, ::-1]
                dst = og[:].rearrange("p (w h) -> p w h", h=H)
                nc.vector.tensor_copy(dst, src)

                # exactly one of these fires for an active unit
                nc.gpsimd.dma_start(out.ap()[0, u], og[:], cond=c_a)
                og_wrev = og[:].rearrange("p (w h) -> p w h", h=H)[:, ::-1, :]
                nc.gpsimd.dma_start(out.ap()[1, u], og_wrev, cond=c_b)
    nc.compile()
    return nc


_NC_CACHE = None


def _get_program():
    global _NC_CACHE
    if _NC_CACHE is None:
        _NC_CACHE = build_program()
    return _NC_CACHE


def kernel(x, x_flag, y_flag, _trace=False, **trace_kwargs):
    x = np.asarray(x)
    if x.dtype != np.float32:
        x = x.astype(np.float32)
    x_flag = np.asarray(x_flag, dtype=np.float32)
    y_flag = np.asarray(y_flag, dtype=np.float32)
    n = x.shape[0]
    assert n == FULL_B, x.shape
    sample_shape = x.shape[1:]
    xv = x.reshape(n, IMG, WH)

    # host schedule: 4 quarter-units per active sample, dealt round-robin
    active = [int(i) for i in np.nonzero(x_flag > 0.5)[0]]
    units = [(idx, u) for idx in active for u in range(UPS)]
    assert len(units) <= N_CORES * U

    xs = np.zeros((N_CORES, U, IMG, CH), dtype=np.float32)
    masks = np.zeros((N_CORES, 3), dtype=np.int32)  # act, a, b bit-masks
    placement = []  # (core, region, slot, sample, out_unit)
    for i, (idx, u) in enumerate(units):
        c, s = i % N_CORES, i // N_CORES
        xs[c, s] = xv[idx, :, u * CH : (u + 1) * CH]
        masks[c, 0] |= 1 << s
        if y_flag[idx] > 0.5:
            masks[c, 2] |= 1 << s
            placement.append((c, 1, s, idx, UPS - 1 - u))
        else:
            masks[c, 1] |= 1 << s
            placement.append((c, 0, s, idx, u))

    in_maps = [{"x": xs[c], "flag_masks": masks[c]} for c in range(N_CORES)]
    nc = _get_program()
    res = run_bass_kernel_spmd(
        nc, in_maps, core_ids=list(range(N_CORES)), trace=_trace, **trace_kwargs
    )

    out = np.zeros((n, IMG, WH), dtype=np.float16)
    for c, r, s, idx, pu in placement:
        out[idx, :, pu * CH : (pu + 1) * CH] = res.results[c]["out"][r, s]
    out = out.reshape((n,) + sample_shape)
    if _trace:
        return out, res
    return out


# revision 26
# speedup vs baseline: 1.3783x; 1.0906x over previous
"""Trainium2 Bass kernel for nn_Mirror: per-sample conditional flips + fp16 cast.

Full op: x [16,2,64,128,128] f32, x_flag [16], y_flag [16] f32 ->
out [16,2,64,128,128] f16 where per sample b:
  out[b] = 0                 if x_flag[b] <= 0.5
         = flip_h(x[b])      if x_flag[b] > 0.5 and y_flag[b] <= 0.5
         = flip_hw(x[b])     if x_flag[b] > 0.5 and y_flag[b] > 0.5

A sample [2,64,128,128] is 128 images of 128x128; images map onto the 128
SBUF partitions, so a sample is a [128, 16384] block whose flips are pure
free-dim manipulations: flip_h reverses within each 128-elem image row;
flip_hw additionally maps w-row block q of the input to block 127-q of the
output.

Device program (identical on all 8 cores): 8 independent UNIT slots, each a
contiguous [128, 4096] f32 block (a quarter sample = 32 w-rows of every
image):
    load  T <- x_unit[u]           (sync HWDGE,   cond = act bit u)
    O = revh(T) cast fp16          (single 1-input DVE pass)
    store out_unit[u] <- O         (gpsimd SWDGE, cond = a bit u)   OR
    store out_unit[u] <- revw(O)   (gpsimd SWDGE, cond = b bit u;
                                    w-block reversal on the SBUF read AP)
Unit flags arrive bit-packed in one int32 per mask (act/a/b), loaded once
into one register per DMA-issuing engine; per-unit conds are pure register
ALU.  Skipped DMAs still bump their semaphores, so the Tile schedule is
flag-oblivious.  Inactive units move zero bytes.

Host side: only ACTIVE samples (x_flag > 0.5) produce units; their 4
quarter-units are dealt round-robin over cores (4k units spread over 8
cores; worst case k=16 -> 8 units/core = 2 full samples).  For y-flipped
samples the host ships quarter q and places the device result at quarter
3-q of the output; inactive samples are host-side zeros.
"""

import numpy as np

import concourse.bass as bass
import concourse.mybir as mybir
import concourse.tile as tile
from concourse import bacc
from concourse.bass_utils import run_bass_kernel_spmd
from concourse.ordered_set import OrderedSet

N_CORES = 8
FULL_B = 16
IMG = 128              # images per sample = SBUF partitions
WH = 16384             # free elems per image
CH = 4096              # unit width (32 w-rows)
UPS = WH // CH         # units per sample (4)
U = 8                  # unit slots per core
H = 128

SP = mybir.EngineType.SP
POOL = mybir.EngineType.Pool


def build_program(sim_init=False):
    nc = bacc.Bacc("TRN2", target_bir_lowering=False, debug=False)
    x = nc.dram_tensor("x", [U, IMG, CH], mybir.dt.float32, kind="ExternalInput")
    # bit-packed unit flags: [act_mask, a_mask, b_mask], bit u = unit u
    fm = nc.dram_tensor("flag_masks", [3], mybir.dt.int32, kind="ExternalInput")
    # two disjoint regions (A: no w-flip, B: w-flipped) so the per-unit
    # conditional stores have no DRAM write-write overlap to serialize on;
    # the host reads whichever region the unit's flag selected
    out = nc.dram_tensor(
        "out", [2, U, IMG, CH], mybir.dt.float16, kind="ExternalOutput"
    )

    with tile.TileContext(nc) as tc:
        with (
            tc.tile_pool(name="flags", bufs=1) as flag_pool,
            tc.tile_pool(name="in", bufs=7) as in_pool,
            tc.tile_pool(name="out", bufs=6) as out_pool,
        ):
            fmt = flag_pool.tile([1, 3], mybir.dt.int32, tag="fm")
            nc.sync.dma_start(fmt[:], fm.ap().unsqueeze(0))

            # one mask register per issuing engine; per-unit conds are pure
            # ALU on the snapped value (no per-unit loads or sem waits)
            actr = nc.alloc_registers("actr", engines=OrderedSet([SP]))
            ar = nc.alloc_registers("ar", engines=OrderedSet([POOL]))
            br = nc.alloc_registers("br", engines=OrderedSet([POOL]))
            nc.regs_load(actr, fmt[0:1, 0:1])
            nc.regs_load(ar, fmt[0:1, 1:2])
            nc.regs_load(br, fmt[0:1, 2:3])
            lim = (1 << U) - 1
            act_m = nc.snap(actr, engines=OrderedSet([SP]), min_val=0, max_val=lim)
            a_m = nc.snap(ar, engines=OrderedSet([POOL]), min_val=0, max_val=lim)
            b_m = nc.snap(br, engines=OrderedSet([POOL]), min_val=0, max_val=lim)

            for u in range(U):
                act = (act_m & (1 << u)) != 0
                c_a = (a_m & (1 << u)) != 0
                c_b = (b_m & (1 << u)) != 0

                t = in_pool.tile([IMG, CH], mybir.dt.float32, tag="tin")
                if sim_init:
                    # CoreSim-only: skipped loads leave tiles uninit, which
                    # the sim rejects; HW reads garbage that is never stored.
                    nc.gpsimd.memset(t[:], 0.0)
                nc.sync.dma_start(t[:], x.ap()[u], cond=act)

                og = out_pool.tile([IMG, CH], mybir.dt.float16, tag="og")
                # O = revh(T): reverse within each 128-elem image row + cast.
                # Even slots on DVE, odd on ACT: the host fills even slots
                # first, so real casts land on the faster DVE while dead
                # casts run early on ACT, and neither serializes the other.
                src = t[:].rearrange("p (w h) -> p w h", h=H)[:, :, ::-1]
                dst = og[:].rearrange("p (w h) -> p w h", h=H)
                if u % 2 == 0:
                    nc.vector.tensor_copy(dst, src)
                else:
                    nc.scalar.copy(dst, src)

                # exactly one of these fires for an active unit
                nc.gpsimd.dma_start(out.ap()[0, u], og[:], cond=c_a)
                og_wrev = og[:].rearrange("p (w h) -> p w h", h=H)[:, ::-1, :]
                nc.gpsimd.dma_start(out.ap()[1, u], og_wrev, cond=c_b)
    nc.compile()
    return nc


_NC_CACHE = None


def _get_program():
    global _NC_CACHE
    if _NC_CACHE is None:
        _NC_CACHE = build_program()
    return _NC_CACHE


def kernel(x, x_flag, y_flag, _trace=False, **trace_kwargs):
    x = np.asarray(x)
    if x.dtype != np.float32:
        x = x.astype(np.float32)
    x_flag = np.asarray(x_flag, dtype=np.float32)
    y_flag = np.asarray(y_flag, dtype=np.float32)
    n = x.shape[0]
    assert n == FULL_B, x.shape
    sample_shape = x.shape[1:]
    xv = x.reshape(n, IMG, WH)

    # host schedule: 4 quarter-units per active sample, dealt round-robin
    active = [int(i) for i in np.nonzero(x_flag > 0.5)[0]]
    units = [(idx, u) for idx in active for u in range(UPS)]
    assert len(units) <= N_CORES * U

    xs = np.zeros((N_CORES, U, IMG, CH), dtype=np.float32)
    masks = np.zeros((N_CORES, 3), dtype=np.int32)  # act, a, b bit-masks
    placement = []  # (core, region, slot, sample, out_unit)
    # even slots first: dead compute for empty slots interleaves with real
    # work in the device's unit order instead of padding the kernel tail
    slot_order = [s for s in range(U) if s % 2 == 0] + [
        s for s in range(U) if s % 2 == 1
    ]
    for i, (idx, u) in enumerate(units):
        c, s = i % N_CORES, slot_order[i // N_CORES]
        xs[c, s] = xv[idx, :, u * CH : (u + 1) * CH]
        masks[c, 0] |= 1 << s
        if y_flag[idx] > 0.5:
            masks[c, 2] |= 1 << s
            placement.append((c, 1, s, idx, UPS - 1 - u))
        else:
            masks[c, 1] |= 1 << s
            placement.append((c, 0, s, idx, u))

    in_maps = [{"x": xs[c], "flag_masks": masks[c]} for c in range(N_CORES)]
    nc = _get_program()
    res = run_bass_kernel_spmd(
        nc, in_maps, core_ids=list(range(N_CORES)), trace=_trace, **trace_kwargs
    )

    out = np.zeros((n, IMG, WH), dtype=np.float16)
    for c, r, s, idx, pu in placement:
        out[idx, :, pu * CH : (pu + 1) * CH] = res.results[c]["out"][r, s]
    out = out.reshape((n,) + sample_shape)
    if _trace:
        return out, res
    return out


# revision 27
# speedup vs baseline: 1.4340x; 1.0404x over previous
"""Trainium2 Bass kernel for nn_Mirror: per-sample conditional flips + fp16 cast.

Full op: x [16,2,64,128,128] f32, x_flag [16], y_flag [16] f32 ->
out [16,2,64,128,128] f16 where per sample b:
  out[b] = 0                 if x_flag[b] <= 0.5
         = flip_h(x[b])      if x_flag[b] > 0.5 and y_flag[b] <= 0.5
         = flip_hw(x[b])     if x_flag[b] > 0.5 and y_flag[b] > 0.5

A sample [2,64,128,128] is 128 images of 128x128; images map onto the 128
SBUF partitions, so a sample is a [128, 16384] block whose flips are pure
free-dim manipulations: flip_h reverses within each 128-elem image row;
flip_hw additionally maps w-row block q of the input to block 127-q of the
output.

Device program (identical on all 8 cores): 8 independent UNIT slots, each a
contiguous [128, 4096] f32 block (a quarter sample = 32 w-rows of every
image):
    load  T <- x_unit[u]           (sync HWDGE,   cond = act bit u)
    O = revh(T) cast fp16          (single 1-input DVE pass)
    store out_unit[u] <- O         (gpsimd SWDGE, cond = a bit u)   OR
    store out_unit[u] <- revw(O)   (gpsimd SWDGE, cond = b bit u;
                                    w-block reversal on the SBUF read AP)
Unit flags arrive bit-packed in one int32 per mask (act/a/b), loaded once
into one register per DMA-issuing engine; per-unit conds are pure register
ALU.  Skipped DMAs still bump their semaphores, so the Tile schedule is
flag-oblivious.  Inactive units move zero bytes.

Host side: only ACTIVE samples (x_flag > 0.5) produce units; their 4
quarter-units are dealt round-robin over cores (4k units spread over 8
cores; worst case k=16 -> 8 units/core = 2 full samples).  For y-flipped
samples the host ships quarter q and places the device result at quarter
3-q of the output; inactive samples are host-side zeros.
"""

import numpy as np

import concourse.bass as bass
import concourse.mybir as mybir
import concourse.tile as tile
from concourse import bacc
from concourse.bass_utils import run_bass_kernel_spmd
from concourse.ordered_set import OrderedSet

N_CORES = 8
FULL_B = 16
IMG = 128              # images per sample = SBUF partitions
WH = 16384             # free elems per image
CH = 4096              # unit width (32 w-rows)
UPS = WH // CH         # units per sample (4)
U = 8                  # unit slots per core
H = 128

SP = mybir.EngineType.SP
POOL = mybir.EngineType.Pool


def build_program(sim_init=False):
    nc = bacc.Bacc("TRN2", target_bir_lowering=False, debug=False)
    x = nc.dram_tensor("x", [U, IMG, CH], mybir.dt.float32, kind="ExternalInput")
    # bit-packed unit flags: [act_mask, a_mask, b_mask], bit u = unit u
    fm = nc.dram_tensor("flag_masks", [3], mybir.dt.int32, kind="ExternalInput")
    # two disjoint regions (A: no w-flip, B: w-flipped) so the per-unit
    # conditional stores have no DRAM write-write overlap to serialize on;
    # the host reads whichever region the unit's flag selected
    out = nc.dram_tensor(
        "out", [2, U, IMG, CH], mybir.dt.float16, kind="ExternalOutput"
    )

    with tile.TileContext(nc) as tc:
        with (
            tc.tile_pool(name="flags", bufs=1) as flag_pool,
            tc.tile_pool(name="in", bufs=7) as in_pool,
            tc.tile_pool(name="out", bufs=6) as out_pool,
        ):
            fmt = flag_pool.tile([1, 3], mybir.dt.int32, tag="fm")
            nc.sync.dma_start(fmt[:], fm.ap().unsqueeze(0))

            # one mask register per issuing engine; per-unit conds are pure
            # ALU on the snapped value (no per-unit loads or sem waits)
            actr = nc.alloc_registers("actr", engines=OrderedSet([SP]))
            ar = nc.alloc_registers("ar", engines=OrderedSet([POOL]))
            br = nc.alloc_registers("br", engines=OrderedSet([POOL]))
            nc.regs_load(actr, fmt[0:1, 0:1])
            nc.regs_load(ar, fmt[0:1, 1:2])
            nc.regs_load(br, fmt[0:1, 2:3])
            lim = (1 << U) - 1
            act_m = nc.snap(actr, engines=OrderedSet([SP]), min_val=0, max_val=lim)
            a_m = nc.snap(ar, engines=OrderedSet([POOL]), min_val=0, max_val=lim)
            b_m = nc.snap(br, engines=OrderedSet([POOL]), min_val=0, max_val=lim)

            for u in range(U):
                act = (act_m & (1 << u)) != 0
                c_a = (a_m & (1 << u)) != 0
                c_b = (b_m & (1 << u)) != 0

                t = in_pool.tile([IMG, CH], mybir.dt.float32, tag="tin")
                if sim_init:
                    # CoreSim-only: skipped loads leave tiles uninit, which
                    # the sim rejects; HW reads garbage that is never stored.
                    nc.gpsimd.memset(t[:], 0.0)
                nc.sync.dma_start(t[:], x.ap()[u], cond=act)

                og = out_pool.tile([IMG, CH], mybir.dt.float16, tag="og")
                # O = revh(T): reverse within each 128-elem image row + cast.
                # Even slots on DVE, odd on ACT: the host fills even slots
                # first, so real casts land on the faster DVE while dead
                # casts run early on ACT, and neither serializes the other.
                src = t[:].rearrange("p (w h) -> p w h", h=H)[:, :, ::-1]
                dst = og[:].rearrange("p (w h) -> p w h", h=H)
                if u % 2 == 0:
                    nc.vector.tensor_copy(dst, src)
                else:
                    nc.scalar.copy(dst, src)

                # exactly one of these fires for an active unit
                nc.gpsimd.dma_start(out.ap()[0, u], og[:], cond=c_a)
                og_wrev = og[:].rearrange("p (w h) -> p w h", h=H)[:, ::-1, :]
                nc.gpsimd.dma_start(out.ap()[1, u], og_wrev, cond=c_b)
    nc.compile()
    return nc


_NC_CACHE = None


def _get_program():
    global _NC_CACHE
    if _NC_CACHE is None:
        _NC_CACHE = build_program()
    return _NC_CACHE


def kernel(x, x_flag, y_flag, _trace=False, **trace_kwargs):
    x = np.asarray(x)
    if x.dtype != np.float32:
        x = x.astype(np.float32)
    x_flag = np.asarray(x_flag, dtype=np.float32)
    y_flag = np.asarray(y_flag, dtype=np.float32)
    n = x.shape[0]
    assert n == FULL_B, x.shape
    sample_shape = x.shape[1:]
    xv = x.reshape(n, IMG, WH)

    # host schedule: 4 quarter-units per active sample, dealt round-robin
    active = [int(i) for i in np.nonzero(x_flag > 0.5)[0]]
    units = [(idx, u) for idx in active for u in range(UPS)]
    assert len(units) <= N_CORES * U

    xs = np.zeros((N_CORES, U, IMG, CH), dtype=np.float32)
    masks = np.zeros((N_CORES, 3), dtype=np.int32)  # act, a, b bit-masks
    placement = []  # (core, region, slot, sample, out_unit)
    # even slots first: dead compute for empty slots interleaves with real
    # work in the device's unit order instead of padding the kernel tail
    slot_order = [s for s in range(U) if s % 2 == 0] + [
        s for s in range(U) if s % 2 == 1
    ]
    # HBM is shared between NeuronCore pairs: deal cores in an order that
    # balances unit counts across pairs for both (2i,2i+1) and (i,i+4)
    # pairings, so no HBM domain carries more than its share
    core_order = [0, 2, 5, 7, 1, 3, 4, 6]
    for i, (idx, u) in enumerate(units):
        c, s = core_order[i % N_CORES], slot_order[i // N_CORES]
        xs[c, s] = xv[idx, :, u * CH : (u + 1) * CH]
        masks[c, 0] |= 1 << s
        if y_flag[idx] > 0.5:
            masks[c, 2] |= 1 << s
            placement.append((c, 1, s, idx, UPS - 1 - u))
        else:
            masks[c, 1] |= 1 << s
            placement.append((c, 0, s, idx, u))

    in_maps = [{"x": xs[c], "flag_masks": masks[c]} for c in range(N_CORES)]
    nc = _get_program()
    res = run_bass_kernel_spmd(
        nc, in_maps, core_ids=list(range(N_CORES)), trace=_trace, **trace_kwargs
    )

    out = np.zeros((n, IMG, WH), dtype=np.float16)
    for c, r, s, idx, pu in placement:
        out[idx, :, pu * CH : (pu + 1) * CH] = res.results[c]["out"][r, s]
    out = out.reshape((n,) + sample_shape)
    if _trace:
        return out, res
    return out
